# revision 13
# baseline (speedup 1.0000x reference)
"""Trainium2 Bass kernel for nn_KANSplineLayer (KAN spline layer, 8-core SPMD).

Math rewrite (validated): the 9-triangle spline per (o,i) is piecewise-linear
in t = 4*minmax(x) over [0,4], so it equals a combination of
{t, relu(t-1), relu(t-2), relu(t-3), 1}.

v2 restructure (fp16 GEMMs, PSUM fp32):
  - host pre-transposes x (no DMA XBAR transpose): device does plain
    contiguous DMA in 4 pieces at full HBM rate.
  - min/max via chained tensor_tensor min/max per DMA piece + one final
    tensor_reduce per (block, op), pipelined behind the input DMA pieces ->
    collective doorbell fires much earlier than v1.  (tensor_tensor_reduce
    hangs TRN2 here -- bisected on HW; do not reintroduce it.)
  - s4 folded into the relu-plane WEIGHTS post-collective (6 small
    tensor_scalar ops): relu planes relu(x - (gmin + m*qt)) come directly
    from raw x (relu(t-m) == s4*relu(x-cm)), so only the t-plane needs a
    per-chunk scaled operand tpl=(x-gmin)*s4.
  - epilogue is one tensor_tensor add: out = psum + (silu_base + C_s);
    C_s is folded into silu_base during the collective-wait window.
  - base path (GEMM+BN+SiLU) runs during the collective wait window.
  - spline phase measured at 263ns/MM median issue gap with zero >800ns
    stalls (fp16 N=512 roofline); HW exec 119.7us vs 136.7us v1 baseline.
"""
import numpy as np

import concourse.bacc as bacc
import concourse.bass as bass
import concourse.tile as tile
from concourse import mybir
from concourse.bass_utils import run_bass_kernel_spmd

# ---- problem constants (hardcoded; kernel.py must be self-contained) ----
IN_F, OUT_F = 256, 256
K_KNOTS = 9
EPS_MINMAX = 1e-7
EPS_BN = 1e-3
B, H, W = 32, 32, 32
N_TOTAL = B * H * W            # 32768 rows
N_CORES = 8
N_SHARD = N_TOTAL // N_CORES   # 4096 rows per core
CH = 512                       # spline row chunk (moving dim)
N_CHUNKS = N_SHARD // CH       # 8
N_PIECE = 4                    # input DMA pieces
PIECE = N_SHARD // N_PIECE     # 1024 rows per piece

F32 = mybir.dt.float32
F16 = mybir.dt.float16


def _host_prep(base_weight, spline_weight, spline_scaler,
               bn_base_gamma, bn_base_beta, bn_base_mean, bn_base_var,
               bn_spline_gamma, bn_spline_beta, bn_spline_mean, bn_spline_var):
    """Fold BN + rewrite spline into relu-plane weights. All in float64.

    Returns SBUF-layout arrays:
      w_t  [128, 2, 2, 128]    (i', b, bo, o')  t-plane weights (unscaled)
      w_r  [128, 3, 2, 2, 128] (i', m, b, bo, o') relu-plane weights
      w_b  [128, 2, 2, 128]    base weights (BN folded)
      cs   [128, 2]            spline constant C_s (bo-major), f32
      bb   [128, 2]            base bias per (o', bo), f32
    """
    f64 = np.float64
    w = np.asarray(spline_weight, f64) * np.asarray(spline_scaler, f64)[:, :, None]
    knots = np.linspace(-1.0, 1.0, K_KNOTS).astype(f64)
    jg = np.arange(5, dtype=f64) / 4.0
    tri = np.maximum(0.0, 1.0 - np.abs(jg[None, :] - knots[:, None]))   # [k, j]
    G = np.einsum('oik,kj->oij', w, tri)                                # [o,i,5]
    a_s = np.asarray(bn_spline_gamma, f64) / np.sqrt(np.asarray(bn_spline_var, f64) + EPS_BN)
    b_s = np.asarray(bn_spline_beta, f64) - a_s * np.asarray(bn_spline_mean, f64)
    G = G * a_s[:, None, None]
    W_t = (G[:, :, 1] - G[:, :, 0]).T                                   # [i,o]
    Hs = [(G[:, :, 2] - 2 * G[:, :, 1] + G[:, :, 0]).T,
          (G[:, :, 3] - 2 * G[:, :, 2] + G[:, :, 1]).T,
          (G[:, :, 4] - 2 * G[:, :, 3] + G[:, :, 2]).T]                 # [i,o]
    C_s = G[:, :, 0].sum(axis=1) + b_s                                  # [o]
    a_b = np.asarray(bn_base_gamma, f64) / np.sqrt(np.asarray(bn_base_var, f64) + EPS_BN)
    b_b = np.asarray(bn_base_beta, f64) - a_b * np.asarray(bn_base_mean, f64)
    Wb = np.asarray(base_weight, f64) * a_b[None, :]                    # [i,o]

    def blk(M):  # [in, out] f64 -> [128, 2(b), 2(bo), 128] (i', b, bo, o')
        return M.reshape(2, 128, 2, 128).transpose(1, 0, 2, 3)

    w_t = blk(W_t).astype(np.float16)
    w_b = blk(Wb).astype(np.float16)
    w_r = np.stack([blk(Hm) for Hm in Hs], axis=1).astype(np.float16)  # [128,3,2,2,128]
    cs = C_s.reshape(2, 128).T.astype(np.float32)                      # [128, 2]
    bb = b_b.reshape(2, 128).T.astype(np.float32)                      # [128, 2]
    return w_t, w_r, w_b, cs, bb


def _build_bass():
    nc = bacc.Bacc(num_devices=N_CORES)
    x_d = nc.declare_dram_parameter("x_t", [128, 2, N_SHARD], F16, isOutput=False)
    w_t_d = nc.declare_dram_parameter("w_t", [128, 2, 2, 128], F16, isOutput=False)
    w_r_d = nc.declare_dram_parameter("w_r", [128, 3, 2, 2, 128], F16, isOutput=False)
    w_b_d = nc.declare_dram_parameter("w_b", [128, 2, 2, 128], F16, isOutput=False)
    cs_d = nc.declare_dram_parameter("cs", [128, 2], F32, isOutput=False)
    bb_d = nc.declare_dram_parameter("bb", [128, 2], F32, isOutput=False)
    out_t = nc.declare_dram_parameter("out_t", [2, 128, N_SHARD], F16, isOutput=True)

    from contextlib import ExitStack
    with tile.TileContext(nc) as tc, ExitStack() as es:
        cons = es.enter_context(tc.tile_pool(name="cons", bufs=1))
        silu_p = es.enter_context(tc.tile_pool(name="silu", bufs=1))
        planes_p = es.enter_context(tc.tile_pool(name="planes", bufs=3))
        psS = es.enter_context(tc.tile_pool(name="psS", bufs=4, space="PSUM"))
        psB = es.enter_context(tc.tile_pool(name="psB", bufs=2, space="PSUM"))
        psH = es.enter_context(tc.tile_pool(name="psH", bufs=2, space="PSUM"))
        outp = es.enter_context(tc.tile_pool(name="outp", bufs=4))
        dram = es.enter_context(tc.tile_pool(name="dram", bufs=2, space="DRAM"))

        # ---- input DMA: x^T pieces on sync ring; weights on scalar ring ----
        # pieces are per (block, half): 4 KB contiguous per partition on both
        # sides (line-rate descriptors); order b0j0,b1j0,b0j1,b1j1 so base
        # chunks 0-3 unblock after the 2nd piece.
        xt = cons.tile([128, 2, N_SHARD], F16, name="xt")
        HALF_SH = N_SHARD // 2
        for j in range(2):
            for b in range(2):
                sl = slice(j * HALF_SH, (j + 1) * HALF_SH)
                nc.sync.dma_start(out=xt[:, b, sl], in_=x_d[:, b, sl])

        wb_sb = cons.tile([128, 2, 2, 128], F16, name="wb_sb")
        nc.scalar.dma_start(out=wb_sb[:], in_=w_b_d[:])
        wt_sb = cons.tile([128, 2, 2, 128], F16, name="wt_sb")
        nc.scalar.dma_start(out=wt_sb[:], in_=w_t_d[:])
        wr_sb = cons.tile([128, 3, 2, 2, 128], F16, name="wr_sb")
        nc.scalar.dma_start(out=wr_sb[:], in_=w_r_d[:])
        cs_sb = cons.tile([128, 2], F32, name="cs_sb")
        nc.scalar.dma_start(out=cs_sb[:], in_=cs_d[:])
        bb_sb = cons.tile([128, 2], F32, name="bb_sb")
        nc.scalar.dma_start(out=bb_sb[:], in_=bb_d[:])

        # ---- local min/max: per-piece 2048->1024 folds (DMA-pipelined),
        # then combine/fold/reduce trees per (block, op) on DVE ----
        g_min = cons.tile([128, 2, 2, 1024], F16, name="g_min")  # (b, j)
        g_max = cons.tile([128, 2, 2, 1024], F16, name="g_max")
        for j in range(2):
            for b in range(2):
                lo = xt[:, b, j * HALF_SH:j * HALF_SH + 1024]
                hi = xt[:, b, j * HALF_SH + 1024:(j + 1) * HALF_SH]
                nc.vector.tensor_tensor(
                    out=g_min[:, b, j, :], in0=lo, in1=hi,
                    op=mybir.AluOpType.min)
                nc.vector.tensor_tensor(
                    out=g_max[:, b, j, :], in0=lo, in1=hi,
                    op=mybir.AluOpType.max)
        h_min = cons.tile([128, 2, 1024], F16, name="h_min")
        h_max = cons.tile([128, 2, 1024], F16, name="h_max")
        q_min = cons.tile([128, 2, 512], F16, name="q_min")
        q_max = cons.tile([128, 2, 512], F16, name="q_max")
        mm_loc = cons.tile([128, 4], F16, name="mm_loc")  # [min0,min1,-max0,-max1]
        tmax = cons.tile([128, 2], F16, name="tmax")
        for b in range(2):
            nc.vector.tensor_tensor(
                out=h_min[:, b, :], in0=g_min[:, b, 0, :], in1=g_min[:, b, 1, :],
                op=mybir.AluOpType.min)
            nc.vector.tensor_tensor(
                out=h_max[:, b, :], in0=g_max[:, b, 0, :], in1=g_max[:, b, 1, :],
                op=mybir.AluOpType.max)
        for b in range(2):
            nc.vector.tensor_tensor(
                out=q_min[:, b, :], in0=h_min[:, b, 0:512], in1=h_min[:, b, 512:1024],
                op=mybir.AluOpType.min)
            nc.vector.tensor_tensor(
                out=q_max[:, b, :], in0=h_max[:, b, 0:512], in1=h_max[:, b, 512:1024],
                op=mybir.AluOpType.max)
        for b in range(2):
            nc.vector.tensor_reduce(
                out=mm_loc[:, b:b + 1], in_=q_min[:, b, :],
                op=mybir.AluOpType.min, axis=mybir.AxisListType.X)
            nc.vector.tensor_reduce(
                out=tmax[:, b:b + 1], in_=q_max[:, b, :],
                op=mybir.AluOpType.max, axis=mybir.AxisListType.X)
        nc.vector.tensor_scalar(
            out=mm_loc[:, 2:4], in0=tmax[:], scalar1=-1.0, scalar2=None,
            op0=mybir.AluOpType.mult)

        # ---- global min/max: AllGather of [128,4], 8-way min combine ----
        cc_in = dram.tile([128, 4], F16)
        cc_out = nc.dram_tensor("ag_out", [N_CORES, 128, 4], F16,
                                addr_space="Shared")
        nc.sync.dma_start(out=cc_in[:], in_=mm_loc[:])
        nc.gpsimd.collective_compute(
            "AllGather", mybir.AluOpType.bypass,
            replica_groups=[list(range(N_CORES))],
            ins=[cc_in.opt()], outs=[cc_out[:].opt()])

        # ---- base path (x @ Wb -> BN -> SiLU) fills the collective wait ----
        silu_sb = [[silu_p.tile([128, CH], F16, name=f"silu_{c}_{bo}")
                    for bo in range(2)] for c in range(N_CHUNKS)]
        for c in range(N_CHUNKS):
            cs_sl = slice(c * CH, (c + 1) * CH)
            for bo in range(2):
                pb = psB.tile([128, CH], F32, tag="pb")
                for b in range(2):
                    nc.tensor.matmul(
                        pb[:], wb_sb[:, b, bo, :], xt[:, b, cs_sl],
                        start=(b == 0), stop=(b == 1), skip_group_check=True)
                nc.scalar.activation(
                    out=silu_sb[c][bo][:], in_=pb[:],
                    func=mybir.ActivationFunctionType.Silu,
                    bias=bb_sb[:, bo:bo + 1], scale=1.0)
                # fold the spline constant C_s here (collective-wait window)
                nc.vector.tensor_scalar(
                    out=silu_sb[c][bo][:], in0=silu_sb[c][bo][:],
                    scalar1=cs_sb[:, bo:bo + 1], scalar2=None,
                    op0=mybir.AluOpType.add)

        # ---- collective result -> gmm [gmin0,gmin1,-gmax0,-gmax1] ----
        allg = cons.tile([128, N_CORES, 4], F16, name="allg")
        nc.sync.dma_start(out=allg[:], in_=cc_out[:].rearrange("s p f -> p s f"))
        f4 = cons.tile([128, 4, 4], F16, name="f4")
        f2 = cons.tile([128, 2, 4], F16, name="f2")
        gmm = cons.tile([128, 4], F32, name="gmm")
        nc.vector.tensor_tensor(
            out=f4[:], in0=allg[:, 0:4, :], in1=allg[:, 4:8, :],
            op=mybir.AluOpType.min)
        nc.vector.tensor_tensor(
            out=f2[:], in0=f4[:, 0:2, :], in1=f4[:, 2:4, :],
            op=mybir.AluOpType.min)
        nc.vector.tensor_tensor(
            out=gmm[:], in0=f2[:, 0, :], in1=f2[:, 1, :],
            op=mybir.AluOpType.min)

        # qt = (gmax-gmin+eps)/4 ; s4 = 1/qt
        nrng = cons.tile([128, 2], F32, name="nrng")
        qt = cons.tile([128, 2], F32, name="qt")
        s4 = cons.tile([128, 2], F32, name="s4")
        nc.vector.tensor_tensor(
            out=nrng[:], in0=gmm[:, 0:2], in1=gmm[:, 2:4],
            op=mybir.AluOpType.add)                       # gmin - gmax
        nc.vector.tensor_scalar(
            out=qt[:], in0=nrng[:], scalar1=-0.25, scalar2=EPS_MINMAX * 0.25,
            op0=mybir.AluOpType.mult, op1=mybir.AluOpType.add)
        nc.vector.reciprocal(out=s4[:], in_=qt[:])

        # bneg[m-1, b] = -(gmin + m*qt)   (ACT relu bias / DVE add operand)
        bneg = cons.tile([128, 3, 2], F32, name="bneg")
        for m in (1, 2, 3):
            for b in range(2):
                nc.vector.tensor_scalar(
                    out=bneg[:, m - 1, b:b + 1], in0=qt[:, b:b + 1],
                    scalar1=-float(m), scalar2=gmm[:, b:b + 1],
                    op0=mybir.AluOpType.mult, op1=mybir.AluOpType.subtract)

        # scale relu-plane weights by s4 (per input-feature partition);
        # the t-plane keeps unscaled weights and uses tpl=(x-gmin)*s4.
        # chunk-0 planes are emitted on DVE *before* the weight scaling so
        # the first t-plane matmuls can start as early as possible; the
        # wr_s loop is m-major so the m=0 weights (needed first) land first.
        tpl0 = [planes_p.tile([128, CH], F16, tag=f"t{b}", name=f"t{b}_0")
                for b in range(2)]
        rpl30 = [planes_p.tile([128, CH], F16, tag=f"r2{b}", name=f"r2{b}_0")
                 for b in range(2)]
        for b in range(2):
            nc.vector.tensor_scalar(
                out=tpl0[b][:], in0=xt[:, b, 0:CH],
                scalar1=gmm[:, b:b + 1], scalar2=s4[:, b:b + 1],
                op0=mybir.AluOpType.subtract, op1=mybir.AluOpType.mult)
            nc.vector.tensor_scalar(
                out=rpl30[b][:], in0=xt[:, b, 0:CH],
                scalar1=bneg[:, 2, b:b + 1], scalar2=0.0,
                op0=mybir.AluOpType.add, op1=mybir.AluOpType.max)
        wr_s = cons.tile([128, 3, 2, 2, 128], F16, name="wr_s")
        for m in range(3):
            for b in range(2):
                nc.vector.tensor_scalar(
                    out=wr_s[:, m, b, :, :], in0=wr_sb[:, m, b, :, :],
                    scalar1=s4[:, b:b + 1], scalar2=None,
                    op0=mybir.AluOpType.mult)

        # ---- spline phase: relu planes + 8 GEMMs per (chunk, bo) ----
        for c in range(N_CHUNKS):
            cs_sl = slice(c * CH, (c + 1) * CH)
            # relu planes from raw x: m=1,2 on ACT; m=3 on DVE
            rpl = [[planes_p.tile([128, CH], F16, tag=f"r{m}{b}", name=f"r{m}{b}_{c}")
                    for b in range(2)] if m < 2 else
                   (rpl30 if c == 0 else
                    [planes_p.tile([128, CH], F16, tag=f"r2{b}", name=f"r2{b}_{c}")
                     for b in range(2)])
                   for m in range(3)]
            tpl = (tpl0 if c == 0 else
                   [planes_p.tile([128, CH], F16, tag=f"t{b}", name=f"t{b}_{c}")
                    for b in range(2)])
            for b in range(2):
                if c > 0:
                    nc.vector.tensor_scalar(
                        out=tpl[b][:], in0=xt[:, b, cs_sl],
                        scalar1=gmm[:, b:b + 1], scalar2=s4[:, b:b + 1],
                        op0=mybir.AluOpType.subtract, op1=mybir.AluOpType.mult)
                for m in (1, 2):
                    nc.scalar.activation(
                        out=rpl[m - 1][b][:], in_=xt[:, b, cs_sl],
                        func=mybir.ActivationFunctionType.Relu,
                        bias=bneg[:, m - 1, b:b + 1], scale=1.0)
                if c > 0:
                    nc.vector.tensor_scalar(
                        out=rpl[2][b][:], in0=xt[:, b, cs_sl],
                        scalar1=bneg[:, 2, b:b + 1], scalar2=0.0,
                        op0=mybir.AluOpType.add, op1=mybir.AluOpType.max)
            for bo in range(2):
                # split the very last group so its epilogue/DMA overlaps
                halves = ([(0, CH)] if not (c == N_CHUNKS - 1 and bo == 1)
                          else [(0, CH // 2), (CH // 2, CH)])
                for (h0, h1) in halves:
                    hw = h1 - h0
                    pool = psS if hw == CH else psH
                    ps = pool.tile([128, hw], F32, tag=f"ps{hw}")
                    for b in range(2):
                        nc.tensor.matmul(
                            ps[:], wt_sb[:, b, bo, :], tpl[b][:, h0:h1],
                            start=(b == 0), stop=False, skip_group_check=True)
                    for m in range(3):
                        for b in range(2):
                            nc.tensor.matmul(
                                ps[:], wr_s[:, m, b, bo, :],
                                rpl[m][b][:, h0:h1],
                                start=False, stop=(m == 2 and b == 1),
                                skip_group_check=True)
                    o = outp.tile([128, hw], F16, tag=f"o{hw}")
                    nc.vector.tensor_tensor(
                        out=o[:], in0=ps[:], in1=silu_sb[c][bo][:, h0:h1],
                        op=mybir.AluOpType.add)
                    nc.sync.dma_start(
                        out=out_t[bo, :, c * CH + h0:c * CH + h1], in_=o[:])
    nc.compile()
    return nc


_CACHE = {}


def make_in_maps(inputs):
    x = np.asarray(inputs["x"], np.float32).reshape(N_TOTAL, IN_F)
    x16 = x.astype(np.float16)
    w_t, w_r, w_b, cs, bb = _host_prep(
        **{k: v for k, v in inputs.items() if k != "x"})
    maps = []
    for c in range(N_CORES):
        xs = x16[c * N_SHARD:(c + 1) * N_SHARD]              # [4096, 256]
        xt = np.ascontiguousarray(
            xs.T.reshape(2, 128, N_SHARD).transpose(1, 0, 2))  # [128,2,4096]
        maps.append({
            "x_t": xt,
            "w_t": w_t, "w_r": w_r, "w_b": w_b, "cs": cs, "bb": bb,
        })
    return maps


def kernel(**inputs):
    if "nc" not in _CACHE:
        _CACHE["nc"] = _build_bass()
    nc = _CACHE["nc"]
    in_maps = make_in_maps(inputs)
    res = run_bass_kernel_spmd(nc, in_maps, list(range(N_CORES)))
    out = np.empty((N_TOTAL, OUT_F), np.float32)
    for c in range(N_CORES):
        ot = np.asarray(res.results[c]["out_t"], np.float32)  # [2,128,4096]
        out[c * N_SHARD:(c + 1) * N_SHARD] = (
            ot.transpose(2, 0, 1).reshape(N_SHARD, OUT_F))
    return out.reshape(B, H, W, OUT_F)
